# revision 8
# baseline (speedup 1.0000x reference)
"""MiniAttentionQHead Trainium2 kernel (8-core data parallel), v3 all-bf16.

Same algebra as the f32r baseline (see kernel_f32r_bak.py docstring):
8 distinct kv tokens per row (hidden + 7 untouched ctx slots, hidden's
softmax term counted twice), w_out folded into Wv host-side (U, 32 cols).

v3 changes vs baseline:
  - all PE operands bf16: same 1 cyc/row matmul rate as f32r but moving
    operands can be 1024 wide (half the instructions), FWL applies to the
    128-col stationary loads, and the narrow U-projection matmul runs at
    1 cyc/row instead of f32r's 4 (<256-col penalty).
  - Q phase streams wq chunk-by-chunk (chunk-outer, tiles-inner) so each
    chunk is DMA'd once per pass and its DMA hides under 4 tiles of
    matmul; the wk|u weight DMA for the kv phase overlaps the Q phase.
  - Host layouts are partition-major so every DMA is a straight
    contiguous copy (2-16KB per partition line).
  - DMA bytes halved (bf16).
"""

import math

import numpy as np
import ml_dtypes

B, H, NH, W, A = 4096, 2048, 16, 8, 2
D = H // NH  # 128
NCORES = 8
R = B // NCORES  # 512 rows per core
NT = R // 128  # 4 row tiles
KC = H // 128  # 16 contraction chunks
PASSES = 2
HPP = NH // PASSES  # 8 heads per pass
CW = HPP * D  # 1024 k-channels per pass
UC = HPP * A  # 16 folded-v channels per pass
NTOK = W  # 8 distinct kv tokens

BF16 = ml_dtypes.bfloat16

_cache = {}


def _patch_tile_framework():
    """This environment's walrus accepts only ONE semaphore wait per
    instruction; Tile attaches several.  Patch the end-of-kernel drain and
    add a post-pass that hoists excess waits onto preceding same-engine
    NOPs (engine queues execute sequentially, so semantics are identical).
    """
    import concourse.tile as tile
    from concourse import mybir
    from concourse.vector_clock import ScopedClock

    if getattr(tile.TileContext, "_ant_drain_patched", False):
        return

    def patched(self, tick_clock, wait_clock):
        drain_inst = self.nc.sync.drain()
        wait_clock.add_sem_waits(
            drain_inst.ins, ScopedClock({None: tick_clock.global_clock})
        )
        si = drain_inst.ins.sync_info
        waits = list(si.on_wait or [])
        if len(waits) > 1:
            si.on_wait = waits[:1]
            for w in waits[1:]:
                extra = self.nc.sync.drain()
                extra.ins.sync_info = mybir.SyncInfo(on_wait=[w], on_update=[])
        self.nc.all_engine_barrier()
        assert self.sems is not None
        popped = self.nc._tile_sem_poison_stack.pop()
        assert popped is self._sem_poison
        self.nc.clear_and_free_semaphores(list(self.sems.allocated().values()))
        self.nc.all_engine_barrier()

    tile.TileContext._drain_and_barrier = patched
    tile.TileContext._ant_drain_patched = True


def _dedup_ldweights(nc):
    """Legalization splits every InstMatmult into InstLdweights+InstMatmult
    with no reuse analysis, so back-to-back matmuls sharing one stationary
    operand reload the PE array each time.  Deleting a redundant reload is
    safe: the array holds a copy of the weights, matmuls don't clobber it,
    and (measured) the redundant loads carry no semaphore waits/updates.
    """
    for fn in nc.m.functions:
        for bb in fn.blocks:
            prev = None
            out = []
            for inst in bb.instructions:
                cn = inst.__class__.__name__
                if cn == "InstLdweights":
                    si = inst.sync_info
                    key = (
                        str(inst.ins[0]),
                        str(getattr(inst, "perf_mode", None)),
                        str(getattr(inst, "is_transpose", None)),
                        str(getattr(inst, "tile_position", None)),
                        str(getattr(inst, "tile_size", None)),
                    )
                    clean = si is None or not (si.on_wait or si.on_update)
                    if key == prev and clean:
                        continue  # drop redundant reload
                    prev = key
                elif cn != "InstMatmult" and str(inst.engine).endswith("PE"):
                    prev = None  # unknown PE instruction: stop tracking
                out.append(inst)
            bb.instructions = out


def _split_waits(nc, max_waits=1):
    from concourse import mybir

    cnt = 0
    for fn in nc.m.functions:
        for bb in fn.blocks:
            changed = False
            out = []
            for inst in bb.instructions:
                si = inst.sync_info
                if si is not None:
                    waits = list(si.on_wait or [])
                    if len(waits) > max_waits:
                        extra = waits[:-max_waits]
                        for k in range(0, len(extra), max_waits):
                            nop = mybir.InstNoOp(
                                name=f"I-antws-{cnt}", ins=[], outs=[]
                            )
                            cnt += 1
                            nop.engine = inst.engine
                            nop.sync_info = mybir.SyncInfo(
                                on_wait=extra[k : k + max_waits], on_update=[]
                            )
                            out.append(nop)
                        inst.sync_info = mybir.SyncInfo(
                            on_wait=waits[-max_waits:],
                            on_update=list(si.on_update or []),
                        )
                        changed = True
                out.append(inst)
            if changed:
                bb.instructions = out
    return nc


def _build_nc(reps=1):
    key = ("nc", reps)
    if key in _cache:
        return _cache[key]

    import concourse.bass as bass
    import concourse.tile as tile
    from concourse import mybir

    _patch_tile_framework()

    f32 = mybir.dt.float32
    bf = mybir.dt.bfloat16
    X = mybir.AxisListType.X
    XY = mybir.AxisListType.XY
    ADD = mybir.AluOpType.add
    MAX = mybir.AluOpType.max

    nc = bass.Bass(target_bir_lowering=False)

    hid_d = nc.dram_tensor("hidT", [128, KC, R], bf, kind="ExternalInput")
    ctx_d = nc.dram_tensor(
        "ctxT", [W - 1, NT, 128, KC, 128], bf, kind="ExternalInput"
    )
    wku_d = nc.dram_tensor(
        "wkuT", [PASSES, 128, KC, CW + UC], bf, kind="ExternalInput"
    )
    wq_d = nc.dram_tensor("wqT", [PASSES, KC, 128, CW], bf, kind="ExternalInput")
    out_d = nc.dram_tensor("qout", [R, A], f32, kind="ExternalOutput")

    qscale = 1.0 / math.sqrt(D)

    with tile.TileContext(nc) as tc:
        with tc.tile_pool(name="outer", bufs=1) as outer:
            hid_sb = outer.tile([128, KC, R], bf, tag="hidT")
            nc.sync.dma_start(out=hid_sb, in_=hid_d[:, :, :])
            out_sbs = [
                outer.tile([128, A], f32, tag=f"out{t}", name=f"out{t}")
                for t in range(NT)
            ]

            for _rep in range(reps):
              for pp in range(PASSES):
                with (
                    tc.tile_pool(name=f"res{pp}", bufs=1) as res,
                    tc.tile_pool(name=f"wqs{pp}", bufs=4) as wqs,
                    tc.tile_pool(name=f"ctx{pp}", bufs=4) as ctxp,
                    tc.tile_pool(name=f"prod{pp}", bufs=3) as prodp,
                ):
                    wku_sb = res.tile([128, KC, CW + UC], bf, tag="wku")
                    for c4 in range(4):
                        nc.sync.dma_start(
                            out=wku_sb[:, 4 * c4 : 4 * c4 + 4, :],
                            in_=wku_d[pp, :, 4 * c4 : 4 * c4 + 4, :],
                        )
                    q_sbs = [
                        res.tile([128, CW], f32, tag=f"q{t}", name=f"q{t}")
                        for t in range(NT)
                    ]
                    sc_sbs = [
                        res.tile([128, HPP, NTOK], f32, tag=f"sc{t}", name=f"sc{t}")
                        for t in range(NT)
                    ]
                    vp_sbs = [
                        res.tile([128, NTOK, UC], f32, tag=f"vp{t}", name=f"vp{t}")
                        for t in range(NT)
                    ]

                    # ---- Q phase: q = hidden @ Wq.T (this pass's head half)
                    qps_ctx = tc.tile_pool(name=f"qps{pp}", bufs=NT, space="PSUM")
                    qps = qps_ctx.__enter__()
                    q_ps = [
                        qps.tile([128, CW], f32, tag="qps", name=f"qps{t}")
                        for t in range(NT)
                    ]
                    for c in range(KC):
                        wq_sb = wqs.tile([128, CW], bf, tag="wq")
                        nc.sync.dma_start(out=wq_sb, in_=wq_d[pp, c, :, :])
                        for t in range(NT):
                            lhs = hid_sb[:, c, t * 128 : (t + 1) * 128]
                            for b2 in range(CW // 512):
                                nc.tensor.matmul(
                                    q_ps[t][:, b2 * 512 : (b2 + 1) * 512],
                                    lhs,
                                    wq_sb[:, b2 * 512 : (b2 + 1) * 512],
                                    start=(c == 0),
                                    stop=(c == KC - 1),
                                )
                    for t in range(NT):
                        # PSUM -> SBUF, folding in the 1/sqrt(D) score scale
                        nc.scalar.activation(
                            out=q_sbs[t],
                            in_=q_ps[t],
                            func=mybir.ActivationFunctionType.Copy,
                            scale=qscale,
                        )
                    qps_ctx.__exit__(None, None, None)

                    # ---- KV phase: per (token, tile): k-proj + u-proj + dots
                    with (
                        tc.tile_pool(name=f"kvps{pp}", bufs=3, space="PSUM") as kvps,
                        tc.tile_pool(name=f"vpps{pp}", bufs=2, space="PSUM") as vpps,
                    ):
                        for t in range(NT):
                            for j in range(NTOK):
                                if j == 0:
                                    tok = None
                                else:
                                    tok = ctxp.tile([128, KC, 128], bf, tag="ctx")
                                    nc.sync.dma_start(
                                        out=tok, in_=ctx_d[j - 1, t, :, :, :]
                                    )
                                kb = kvps.tile([128, CW], f32, tag="kb")
                                vpp = vpps.tile([128, UC], f32, tag="vpp")
                                for c in range(KC):
                                    st = c == 0
                                    sp = c == KC - 1
                                    lhs = (
                                        hid_sb[:, c, t * 128 : (t + 1) * 128]
                                        if j == 0
                                        else tok[:, c, :]
                                    )
                                    for b2 in range(CW // 512):
                                        nc.tensor.matmul(
                                            kb[:, b2 * 512 : (b2 + 1) * 512],
                                            lhs,
                                            wku_sb[:, c, b2 * 512 : (b2 + 1) * 512],
                                            start=st,
                                            stop=sp,
                                        )
                                    nc.tensor.matmul(
                                        vpp,
                                        lhs,
                                        wku_sb[:, c, CW : CW + UC],
                                        start=st,
                                        stop=sp,
                                    )
                                pr = prodp.tile([128, CW], f32, tag="pr")
                                nc.vector.tensor_mul(pr, kb, q_sbs[t])
                                nc.vector.tensor_reduce(
                                    out=sc_sbs[t][:, :, j],
                                    in_=pr.rearrange("p (h d) -> p h d", d=D),
                                    axis=X,
                                    op=ADD,
                                )
                                nc.scalar.activation(
                                    out=vp_sbs[t][:, j, :],
                                    in_=vpp,
                                    func=mybir.ActivationFunctionType.Copy,
                                )

                    # ---- softmax + combine per tile
                    with tc.tile_pool(name=f"sm{pp}", bufs=2) as smp:
                        for t in range(NT):
                            mx = smp.tile([128, HPP], f32, tag=f"m{t}")
                            nc.vector.tensor_reduce(
                                out=mx, in_=sc_sbs[t], axis=X, op=MAX
                            )
                            et = smp.tile([128, HPP, NTOK], f32, tag=f"e{t}")
                            for j in range(NTOK):
                                nc.vector.tensor_sub(
                                    et[:, :, j], sc_sbs[t][:, :, j], mx
                                )
                            nc.scalar.activation(
                                out=et, in_=et, func=mybir.ActivationFunctionType.Exp
                            )
                            s8 = smp.tile([128, HPP], f32, tag=f"s8{t}")
                            nc.vector.tensor_reduce(out=s8, in_=et, axis=X, op=ADD)
                            # hidden token appears twice in the kv list
                            nc.vector.tensor_add(s8, s8, et[:, :, 0])
                            rcp = smp.tile([128, HPP], f32, tag=f"r{t}")
                            nc.vector.reciprocal(rcp, s8)
                            at = smp.tile([128, HPP, NTOK], f32, tag=f"a{t}")
                            for j in range(NTOK):
                                nc.vector.tensor_mul(at[:, :, j], et[:, :, j], rcp)
                            vv = vp_sbs[t].rearrange("p j (h a) -> p h j a", a=A)
                            for a in range(A):
                                tmp = smp.tile([128, HPP, NTOK], f32, tag=f"tm{t}")
                                nc.vector.tensor_mul(tmp, at, vv[:, :, :, a])
                                r1 = smp.tile([128, 1], f32, tag=f"r1{t}")
                                r2 = smp.tile([128, 1], f32, tag=f"r2{t}")
                                nc.vector.tensor_reduce(
                                    out=r1, in_=tmp, axis=XY, op=ADD
                                )
                                nc.vector.tensor_reduce(
                                    out=r2, in_=tmp[:, :, 0], axis=X, op=ADD
                                )
                                nc.vector.tensor_add(r1, r1, r2)
                                if pp == 0:
                                    nc.vector.tensor_copy(
                                        out=out_sbs[t][:, a : a + 1], in_=r1
                                    )
                                else:
                                    nc.vector.tensor_add(
                                        out_sbs[t][:, a : a + 1],
                                        out_sbs[t][:, a : a + 1],
                                        r1,
                                    )

            for t in range(NT):
                nc.sync.dma_start(
                    out=out_d[t * 128 : (t + 1) * 128, :], in_=out_sbs[t]
                )

    _dedup_ldweights(nc)
    _split_waits(nc)
    _cache[key] = nc
    return nc


def _prep_inputs(hidden_state, context_buffer, w_qkv, w_out, b_out, context_ptr):
    """Host-side sharding + layout: bf16 casts, partition-major transposes,
    w_out folded into Wv."""
    hidden_state = np.ascontiguousarray(hidden_state, dtype=np.float32)
    context_buffer = np.ascontiguousarray(context_buffer, dtype=np.float32)
    w_qkv = np.ascontiguousarray(w_qkv, dtype=np.float32)
    w_out = np.ascontiguousarray(w_out, dtype=np.float32)

    ptr = int(context_ptr) % W
    kept = [w for w in range(W) if w != ptr]

    wq = w_qkv[0:H]
    wk = w_qkv[H : 2 * H]
    wv = w_qkv[2 * H : 3 * H]
    # U[(h*A+a), ci] = sum_d w_out[a, h*D+d] * Wv[h*D+d, ci]
    U = (
        np.einsum(
            "ahd,hdc->hac",
            w_out.reshape(A, NH, D).astype(np.float64),
            wv.reshape(NH, D, H).astype(np.float64),
        )
        .reshape(NH * A, H)
        .astype(np.float32)
    )

    # weights, partition-major per pass
    wkuT = np.empty((PASSES, 128, KC, CW + UC), dtype=BF16)
    wqT = np.empty((PASSES, KC, 128, CW), dtype=BF16)
    wkT = wk.T.reshape(KC, 128, H)  # [c, p, n]
    uT = U.T.reshape(KC, 128, NH * A)
    wqTf = wq.T.reshape(KC, 128, H)
    for p_ in range(PASSES):
        wkuT[p_, :, :, 0:CW] = (
            wkT[:, :, p_ * CW : (p_ + 1) * CW].transpose(1, 0, 2).astype(BF16)
        )
        wkuT[p_, :, :, CW : CW + UC] = (
            uT[:, :, p_ * UC : (p_ + 1) * UC].transpose(1, 0, 2).astype(BF16)
        )
        wqT[p_] = wqTf[:, :, p_ * CW : (p_ + 1) * CW].astype(BF16)
    wkuT = np.ascontiguousarray(wkuT)
    wqT = np.ascontiguousarray(wqT)

    in_maps = []
    for core in range(NCORES):
        rows = slice(core * R, (core + 1) * R)
        # hidT [p, c, r]
        hidT = np.ascontiguousarray(
            hidden_state[rows].T.reshape(KC, 128, R).transpose(1, 0, 2)
        ).astype(BF16)
        # ctxT [j, t, p, c, r128]
        ctx = context_buffer[rows][:, kept, :]  # [R, 7, H]
        ctxT = np.ascontiguousarray(
            ctx.transpose(1, 2, 0)  # [7, H, R]
            .reshape(W - 1, KC, 128, NT, 128)
            .transpose(0, 3, 2, 1, 4)  # [j, t, p, c, r]
        ).astype(BF16)
        in_maps.append(dict(hidT=hidT, ctxT=ctxT, wkuT=wkuT, wqT=wqT))
    return in_maps


def kernel(hidden_state, context_buffer, w_qkv, w_out, b_out, context_ptr):
    from concourse.bass_utils import run_bass_kernel_spmd

    nc = _build_nc()
    in_maps = _prep_inputs(
        hidden_state, context_buffer, w_qkv, w_out, b_out, context_ptr
    )
    res = run_bass_kernel_spmd(nc, in_maps, core_ids=list(range(NCORES)))
    out = np.concatenate([r["qout"] for r in res.results], axis=0)
    return (out + np.asarray(b_out, dtype=np.float32)[None, :]).astype(np.float32)


# revision 15
# speedup vs baseline: 1.0358x; 1.0358x over previous
"""MiniAttentionQHead Trainium2 kernel (8-core data parallel), v3 all-bf16.

Same algebra as the f32r baseline (see kernel_f32r_bak.py docstring):
8 distinct kv tokens per row (hidden + 7 untouched ctx slots, hidden's
softmax term counted twice), w_out folded into Wv host-side (U, 32 cols).

v3 changes vs baseline:
  - all PE operands bf16: same 1 cyc/row matmul rate as f32r but moving
    operands can be 1024 wide (half the instructions), FWL applies to the
    128-col stationary loads, and the narrow U-projection matmul runs at
    1 cyc/row instead of f32r's 4 (<256-col penalty).
  - Q phase streams wq chunk-by-chunk (chunk-outer, tiles-inner) so each
    chunk is DMA'd once per pass and its DMA hides under 4 tiles of
    matmul; the wk|u weight DMA for the kv phase overlaps the Q phase.
  - Host layouts are partition-major so every DMA is a straight
    contiguous copy (2-16KB per partition line).
  - DMA bytes halved (bf16).
"""

import math

import numpy as np
import ml_dtypes

B, H, NH, W, A = 4096, 2048, 16, 8, 2
D = H // NH  # 128
NCORES = 8
R = B // NCORES  # 512 rows per core
NT = R // 128  # 4 row tiles
KC = H // 128  # 16 contraction chunks
PASSES = 2
HPP = NH // PASSES  # 8 heads per pass
CW = HPP * D  # 1024 k-channels per pass
UC = HPP * A  # 16 folded-v channels per pass
NTOK = W  # 8 distinct kv tokens

BF16 = ml_dtypes.bfloat16

_cache = {}

# experiment knobs (cost-model A/B only; production defaults)
SKIP_DOTS = False
SKIP_SOFTMAX = False
CTX_REUSE = False


def _patch_tile_framework():
    """This environment's walrus accepts only ONE semaphore wait per
    instruction; Tile attaches several.  Patch the end-of-kernel drain and
    add a post-pass that hoists excess waits onto preceding same-engine
    NOPs (engine queues execute sequentially, so semantics are identical).
    """
    import concourse.tile as tile
    from concourse import mybir
    from concourse.vector_clock import ScopedClock

    if getattr(tile.TileContext, "_ant_drain_patched", False):
        return

    def patched(self, tick_clock, wait_clock):
        drain_inst = self.nc.sync.drain()
        wait_clock.add_sem_waits(
            drain_inst.ins, ScopedClock({None: tick_clock.global_clock})
        )
        si = drain_inst.ins.sync_info
        waits = list(si.on_wait or [])
        if len(waits) > 1:
            si.on_wait = waits[:1]
            for w in waits[1:]:
                extra = self.nc.sync.drain()
                extra.ins.sync_info = mybir.SyncInfo(on_wait=[w], on_update=[])
        self.nc.all_engine_barrier()
        assert self.sems is not None
        popped = self.nc._tile_sem_poison_stack.pop()
        assert popped is self._sem_poison
        self.nc.clear_and_free_semaphores(list(self.sems.allocated().values()))
        self.nc.all_engine_barrier()

    tile.TileContext._drain_and_barrier = patched
    tile.TileContext._ant_drain_patched = True


def _dedup_ldweights(nc):
    """Legalization splits every InstMatmult into InstLdweights+InstMatmult
    with no reuse analysis, so back-to-back matmuls sharing one stationary
    operand reload the PE array each time.  Deleting a redundant reload is
    safe: the array holds a copy of the weights, matmuls don't clobber it,
    and (measured) the redundant loads carry no semaphore waits/updates.
    """
    for fn in nc.m.functions:
        for bb in fn.blocks:
            prev = None
            out = []
            for inst in bb.instructions:
                cn = inst.__class__.__name__
                if cn == "InstLdweights":
                    si = inst.sync_info
                    key = (
                        str(inst.ins[0]),
                        str(getattr(inst, "perf_mode", None)),
                        str(getattr(inst, "is_transpose", None)),
                        str(getattr(inst, "tile_position", None)),
                        str(getattr(inst, "tile_size", None)),
                    )
                    clean = si is None or not (si.on_wait or si.on_update)
                    if key == prev and clean:
                        continue  # drop redundant reload
                    prev = key
                elif cn != "InstMatmult" and str(inst.engine).endswith("PE"):
                    prev = None  # unknown PE instruction: stop tracking
                out.append(inst)
            bb.instructions = out


def _split_waits(nc, max_waits=1):
    from concourse import mybir

    cnt = 0
    for fn in nc.m.functions:
        for bb in fn.blocks:
            changed = False
            out = []
            for inst in bb.instructions:
                si = inst.sync_info
                if si is not None:
                    waits = list(si.on_wait or [])
                    if len(waits) > max_waits:
                        extra = waits[:-max_waits]
                        for k in range(0, len(extra), max_waits):
                            nop = mybir.InstNoOp(
                                name=f"I-antws-{cnt}", ins=[], outs=[]
                            )
                            cnt += 1
                            nop.engine = inst.engine
                            nop.sync_info = mybir.SyncInfo(
                                on_wait=extra[k : k + max_waits], on_update=[]
                            )
                            out.append(nop)
                        inst.sync_info = mybir.SyncInfo(
                            on_wait=waits[-max_waits:],
                            on_update=list(si.on_update or []),
                        )
                        changed = True
                out.append(inst)
            if changed:
                bb.instructions = out
    return nc


def _build_nc(reps=1, dedup=True):
    key = ("nc", reps, dedup, SKIP_DOTS, SKIP_SOFTMAX, CTX_REUSE)
    if key in _cache:
        return _cache[key]

    import concourse.bass as bass
    import concourse.tile as tile
    from concourse import mybir

    _patch_tile_framework()

    f32 = mybir.dt.float32
    bf = mybir.dt.bfloat16
    X = mybir.AxisListType.X
    XY = mybir.AxisListType.XY
    ADD = mybir.AluOpType.add
    MAX = mybir.AluOpType.max

    nc = bass.Bass(target_bir_lowering=False)

    hid_d = nc.dram_tensor("hidT", [128, KC, R], bf, kind="ExternalInput")
    ctx_d = nc.dram_tensor(
        "ctxT", [W - 1, NT, 128, KC, 128], bf, kind="ExternalInput"
    )
    wku_d = nc.dram_tensor(
        "wkuT", [PASSES, 128, KC, CW + 2 * UC], bf, kind="ExternalInput"
    )
    wq_d = nc.dram_tensor("wqT", [PASSES, KC, 128, CW], bf, kind="ExternalInput")
    out_d = nc.dram_tensor("qout", [R, A], f32, kind="ExternalOutput")

    qscale = 1.0 / math.sqrt(D)

    with tile.TileContext(nc) as tc:
        with tc.tile_pool(name="outer", bufs=1) as outer:
            hid_sb = outer.tile([128, KC, R], bf, tag="hidT")
            nc.sync.dma_start(out=hid_sb, in_=hid_d[:, :, :])
            out_sbs = [
                outer.tile([128, A], f32, tag=f"out{t}", name=f"out{t}")
                for t in range(NT)
            ]
            vp_sbs = [
                outer.tile([128, NTOK, 2 * UC], f32, tag=f"vp{t}", name=f"vp{t}")
                for t in range(NT)
            ]

            for _rep in range(reps):
              for pp in range(PASSES):
                with (
                    tc.tile_pool(name=f"res{pp}", bufs=1) as res,
                    tc.tile_pool(name=f"wqs{pp}", bufs=6) as wqs,
                    tc.tile_pool(name=f"ctx{pp}", bufs=6) as ctxp,
                    tc.tile_pool(name=f"prod{pp}", bufs=3) as prodp,
                ):
                    wku_sb = res.tile([128, KC, CW + 2 * UC], bf, tag="wku")
                    for c4 in range(4):
                        nc.sync.dma_start(
                            out=wku_sb[:, 4 * c4 : 4 * c4 + 4, :],
                            in_=wku_d[pp, :, 4 * c4 : 4 * c4 + 4, :],
                        )
                    q_sbs = [
                        res.tile([128, CW], f32, tag=f"q{t}", name=f"q{t}")
                        for t in range(NT)
                    ]
                    sc_sbs = [
                        res.tile([128, HPP, NTOK], f32, tag=f"sc{t}", name=f"sc{t}")
                        for t in range(NT)
                    ]

                    # ---- Q phase: q = hidden @ Wq.T (this pass's head half)
                    qps_ctx = tc.tile_pool(name=f"qps{pp}", bufs=NT, space="PSUM")
                    qps = qps_ctx.__enter__()
                    q_ps = [
                        qps.tile([128, CW], f32, tag="qps", name=f"qps{t}")
                        for t in range(NT)
                    ]
                    for c in range(KC):
                        wq_sb = wqs.tile([128, CW], bf, tag="wq")
                        nc.sync.dma_start(out=wq_sb, in_=wq_d[pp, c, :, :])
                        for t in range(NT):
                            lhs = hid_sb[:, c, t * 128 : (t + 1) * 128]
                            for b2 in range(CW // 512):
                                nc.tensor.matmul(
                                    q_ps[t][:, b2 * 512 : (b2 + 1) * 512],
                                    lhs,
                                    wq_sb[:, b2 * 512 : (b2 + 1) * 512],
                                    start=(c == 0),
                                    stop=(c == KC - 1),
                                )
                    for t in range(NT):
                        # PSUM -> SBUF, folding in the 1/sqrt(D) score scale
                        nc.scalar.activation(
                            out=q_sbs[t],
                            in_=q_ps[t],
                            func=mybir.ActivationFunctionType.Copy,
                            scale=qscale,
                        )
                    qps_ctx.__exit__(None, None, None)

                    # ---- KV phase: per (token, tile): k-proj + u-proj + dots
                    if CTX_REUSE:
                        tok_shared = res.tile([128, KC, 128], bf, tag="tokshared")
                        nc.sync.dma_start(
                            out=tok_shared, in_=ctx_d[0, 0, :, :, :]
                        )
                    with (
                        tc.tile_pool(name=f"kvps{pp}", bufs=3, space="PSUM") as kvps,
                        tc.tile_pool(name=f"vpps{pp}", bufs=2, space="PSUM") as vpps,
                    ):
                        for t in range(NT):
                            for j in range(NTOK):
                                if j == 0:
                                    tok = None
                                elif CTX_REUSE:
                                    tok = tok_shared
                                else:
                                    tok = ctxp.tile([128, KC, 128], bf, tag="ctx")
                                    nc.sync.dma_start(
                                        out=tok, in_=ctx_d[j - 1, t, :, :, :]
                                    )
                                kb = kvps.tile([128, CW], f32, tag="kb")
                                if pp == 0:
                                    vpp = vpps.tile(
                                        [128, 2 * UC], f32, tag="vpp", name="vpp"
                                    )
                                else:
                                    vpp = None
                                for c in range(KC):
                                    st = c == 0
                                    sp = c == KC - 1
                                    lhs = (
                                        hid_sb[:, c, t * 128 : (t + 1) * 128]
                                        if j == 0
                                        else tok[:, c, :]
                                    )
                                    for b2 in range(CW // 512):
                                        nc.tensor.matmul(
                                            kb[:, b2 * 512 : (b2 + 1) * 512],
                                            lhs,
                                            wku_sb[:, c, b2 * 512 : (b2 + 1) * 512],
                                            start=st,
                                            stop=sp,
                                        )
                                    if pp == 0:
                                        nc.tensor.matmul(
                                            vpp,
                                            lhs,
                                            wku_sb[:, c, CW : CW + 2 * UC],
                                            start=st,
                                            stop=sp,
                                        )
                                if not SKIP_DOTS:
                                    pr = prodp.tile([128, CW], f32, tag="pr")
                                    nc.vector.tensor_mul(pr, kb, q_sbs[t])
                                    nc.vector.tensor_reduce(
                                        out=sc_sbs[t][:, :, j],
                                        in_=pr.rearrange("p (h d) -> p h d", d=D),
                                        axis=X,
                                        op=ADD,
                                    )
                                else:
                                    nc.vector.tensor_reduce(
                                        out=sc_sbs[t][:, :, j],
                                        in_=kb.rearrange("p (h d) -> p h d", d=D)[:, :, 0:1],
                                        axis=X,
                                        op=ADD,
                                    )
                                if pp == 0:
                                    nc.scalar.activation(
                                        out=vp_sbs[t][:, j, :],
                                        in_=vpp,
                                        func=mybir.ActivationFunctionType.Copy,
                                    )

                    # ---- softmax + combine per tile
                    with tc.tile_pool(name=f"sm{pp}", bufs=2) as smp:
                        for t in range(NT if not SKIP_SOFTMAX else 0):
                            mx = smp.tile([128, HPP], f32, tag=f"m{t}")
                            nc.vector.tensor_reduce(
                                out=mx, in_=sc_sbs[t], axis=X, op=MAX
                            )
                            et = smp.tile([128, HPP, NTOK], f32, tag=f"e{t}")
                            for j in range(NTOK):
                                nc.vector.tensor_sub(
                                    et[:, :, j], sc_sbs[t][:, :, j], mx
                                )
                            nc.scalar.activation(
                                out=et, in_=et, func=mybir.ActivationFunctionType.Exp
                            )
                            s8 = smp.tile([128, HPP], f32, tag=f"s8{t}")
                            nc.vector.tensor_reduce(out=s8, in_=et, axis=X, op=ADD)
                            # hidden token appears twice in the kv list
                            nc.vector.tensor_add(s8, s8, et[:, :, 0])
                            rcp = smp.tile([128, HPP], f32, tag=f"r{t}")
                            nc.vector.reciprocal(rcp, s8)
                            at = smp.tile([128, HPP, NTOK], f32, tag=f"a{t}")
                            for j in range(NTOK):
                                nc.vector.tensor_mul(at[:, :, j], et[:, :, j], rcp)
                            vv = vp_sbs[t].rearrange(
                                "p j (h a) -> p h j a", a=A
                            )[:, pp * HPP : (pp + 1) * HPP]
                            for a in range(A):
                                tmp = smp.tile([128, HPP, NTOK], f32, tag=f"tm{t}")
                                nc.vector.tensor_mul(tmp, at, vv[:, :, :, a])
                                r1 = smp.tile([128, 1], f32, tag=f"r1{t}")
                                r2 = smp.tile([128, 1], f32, tag=f"r2{t}")
                                nc.vector.tensor_reduce(
                                    out=r1, in_=tmp, axis=XY, op=ADD
                                )
                                nc.vector.tensor_reduce(
                                    out=r2, in_=tmp[:, :, 0], axis=X, op=ADD
                                )
                                nc.vector.tensor_add(r1, r1, r2)
                                if pp == 0:
                                    nc.vector.tensor_copy(
                                        out=out_sbs[t][:, a : a + 1], in_=r1
                                    )
                                else:
                                    nc.vector.tensor_add(
                                        out_sbs[t][:, a : a + 1],
                                        out_sbs[t][:, a : a + 1],
                                        r1,
                                    )

            for t in range(NT):
                nc.sync.dma_start(
                    out=out_d[t * 128 : (t + 1) * 128, :], in_=out_sbs[t]
                )

    if dedup:
        _dedup_ldweights(nc)
    _split_waits(nc)
    _cache[key] = nc
    return nc


def _prep_inputs(hidden_state, context_buffer, w_qkv, w_out, b_out, context_ptr):
    """Host-side sharding + layout: bf16 casts, partition-major transposes,
    w_out folded into Wv."""
    hidden_state = np.ascontiguousarray(hidden_state, dtype=np.float32)
    context_buffer = np.ascontiguousarray(context_buffer, dtype=np.float32)
    w_qkv = np.ascontiguousarray(w_qkv, dtype=np.float32)
    w_out = np.ascontiguousarray(w_out, dtype=np.float32)

    ptr = int(context_ptr) % W
    kept = [w for w in range(W) if w != ptr]

    wq = w_qkv[0:H]
    wk = w_qkv[H : 2 * H]
    wv = w_qkv[2 * H : 3 * H]
    # U[(h*A+a), ci] = sum_d w_out[a, h*D+d] * Wv[h*D+d, ci]
    U = (
        np.einsum(
            "ahd,hdc->hac",
            w_out.reshape(A, NH, D).astype(np.float64),
            wv.reshape(NH, D, H).astype(np.float64),
        )
        .reshape(NH * A, H)
        .astype(np.float32)
    )

    # weights, partition-major per pass
    wkuT = np.zeros((PASSES, 128, KC, CW + 2 * UC), dtype=BF16)
    wqT = np.empty((PASSES, KC, 128, CW), dtype=BF16)
    wkT = wk.T.reshape(KC, 128, H)  # [c, p, n]
    uT = U.T.reshape(KC, 128, NH * A)
    wqTf = wq.T.reshape(KC, 128, H)
    for p_ in range(PASSES):
        wkuT[p_, :, :, 0:CW] = (
            wkT[:, :, p_ * CW : (p_ + 1) * CW].transpose(1, 0, 2).astype(BF16)
        )
        wqT[p_] = wqTf[:, :, p_ * CW : (p_ + 1) * CW].astype(BF16)
    wkuT[0, :, :, CW : CW + 2 * UC] = uT.transpose(1, 0, 2).astype(BF16)
    wkuT = np.ascontiguousarray(wkuT)
    wqT = np.ascontiguousarray(wqT)

    in_maps = []
    for core in range(NCORES):
        rows = slice(core * R, (core + 1) * R)
        # hidT [p, c, r]
        hidT = np.ascontiguousarray(
            hidden_state[rows].T.reshape(KC, 128, R).transpose(1, 0, 2)
        ).astype(BF16)
        # ctxT [j, t, p, c, r128]
        ctx = context_buffer[rows][:, kept, :]  # [R, 7, H]
        ctxT = np.ascontiguousarray(
            ctx.transpose(1, 2, 0)  # [7, H, R]
            .reshape(W - 1, KC, 128, NT, 128)
            .transpose(0, 3, 2, 1, 4)  # [j, t, p, c, r]
        ).astype(BF16)
        in_maps.append(dict(hidT=hidT, ctxT=ctxT, wkuT=wkuT, wqT=wqT))
    return in_maps


def kernel(hidden_state, context_buffer, w_qkv, w_out, b_out, context_ptr):
    from concourse.bass_utils import run_bass_kernel_spmd

    nc = _build_nc()
    in_maps = _prep_inputs(
        hidden_state, context_buffer, w_qkv, w_out, b_out, context_ptr
    )
    res = run_bass_kernel_spmd(nc, in_maps, core_ids=list(range(NCORES)))
    out = np.concatenate([r["qout"] for r in res.results], axis=0)
    return (out + np.asarray(b_out, dtype=np.float32)[None, :]).astype(np.float32)


# revision 16
# speedup vs baseline: 1.0618x; 1.0251x over previous
"""MiniAttentionQHead Trainium2 kernel (8-core data parallel), v3 all-bf16.

Same algebra as the f32r baseline (see kernel_f32r_bak.py docstring):
8 distinct kv tokens per row (hidden + 7 untouched ctx slots, hidden's
softmax term counted twice), w_out folded into Wv host-side (U, 32 cols).

v3 changes vs baseline:
  - all PE operands bf16: same 1 cyc/row matmul rate as f32r but moving
    operands can be 1024 wide (half the instructions), FWL applies to the
    128-col stationary loads, and the narrow U-projection matmul runs at
    1 cyc/row instead of f32r's 4 (<256-col penalty).
  - Q phase streams wq chunk-by-chunk (chunk-outer, tiles-inner) so each
    chunk is DMA'd once per pass and its DMA hides under 4 tiles of
    matmul; the wk|u weight DMA for the kv phase overlaps the Q phase.
  - Host layouts are partition-major so every DMA is a straight
    contiguous copy (2-16KB per partition line).
  - DMA bytes halved (bf16).
"""

import math

import numpy as np
import ml_dtypes

B, H, NH, W, A = 4096, 2048, 16, 8, 2
D = H // NH  # 128
NCORES = 8
R = B // NCORES  # 512 rows per core
NT = R // 128  # 4 row tiles
KC = H // 128  # 16 contraction chunks
PASSES = 2
HPP = NH // PASSES  # 8 heads per pass
CW = HPP * D  # 1024 k-channels per pass
UC = HPP * A  # 16 folded-v channels per pass
NTOK = W  # 8 distinct kv tokens

BF16 = ml_dtypes.bfloat16

_cache = {}

# experiment knobs (cost-model A/B only; production defaults)
SKIP_DOTS = False
SKIP_SOFTMAX = False
CTX_REUSE = False


def _patch_tile_framework():
    """This environment's walrus accepts only ONE semaphore wait per
    instruction; Tile attaches several.  Patch the end-of-kernel drain and
    add a post-pass that hoists excess waits onto preceding same-engine
    NOPs (engine queues execute sequentially, so semantics are identical).
    """
    import concourse.tile as tile
    from concourse import mybir
    from concourse.vector_clock import ScopedClock

    if getattr(tile.TileContext, "_ant_drain_patched", False):
        return

    def patched(self, tick_clock, wait_clock):
        drain_inst = self.nc.sync.drain()
        wait_clock.add_sem_waits(
            drain_inst.ins, ScopedClock({None: tick_clock.global_clock})
        )
        si = drain_inst.ins.sync_info
        waits = list(si.on_wait or [])
        if len(waits) > 1:
            si.on_wait = waits[:1]
            for w in waits[1:]:
                extra = self.nc.sync.drain()
                extra.ins.sync_info = mybir.SyncInfo(on_wait=[w], on_update=[])
        self.nc.all_engine_barrier()
        assert self.sems is not None
        popped = self.nc._tile_sem_poison_stack.pop()
        assert popped is self._sem_poison
        self.nc.clear_and_free_semaphores(list(self.sems.allocated().values()))
        self.nc.all_engine_barrier()

    tile.TileContext._drain_and_barrier = patched
    tile.TileContext._ant_drain_patched = True


def _dedup_ldweights(nc):
    """Legalization splits every InstMatmult into InstLdweights+InstMatmult
    with no reuse analysis, so back-to-back matmuls sharing one stationary
    operand reload the PE array each time.  Deleting a redundant reload is
    safe: the array holds a copy of the weights, matmuls don't clobber it,
    and (measured) the redundant loads carry no semaphore waits/updates.
    """
    for fn in nc.m.functions:
        for bb in fn.blocks:
            prev = None
            out = []
            for inst in bb.instructions:
                cn = inst.__class__.__name__
                if cn == "InstLdweights":
                    si = inst.sync_info
                    key = (
                        str(inst.ins[0]),
                        str(getattr(inst, "perf_mode", None)),
                        str(getattr(inst, "is_transpose", None)),
                        str(getattr(inst, "tile_position", None)),
                        str(getattr(inst, "tile_size", None)),
                    )
                    clean = si is None or not (si.on_wait or si.on_update)
                    if key == prev and clean:
                        continue  # drop redundant reload
                    prev = key
                elif cn != "InstMatmult" and str(inst.engine).endswith("PE"):
                    prev = None  # unknown PE instruction: stop tracking
                out.append(inst)
            bb.instructions = out


def _split_waits(nc, max_waits=1):
    from concourse import mybir

    cnt = 0
    for fn in nc.m.functions:
        for bb in fn.blocks:
            changed = False
            out = []
            for inst in bb.instructions:
                si = inst.sync_info
                if si is not None:
                    waits = list(si.on_wait or [])
                    if len(waits) > max_waits:
                        extra = waits[:-max_waits]
                        for k in range(0, len(extra), max_waits):
                            nop = mybir.InstNoOp(
                                name=f"I-antws-{cnt}", ins=[], outs=[]
                            )
                            cnt += 1
                            nop.engine = inst.engine
                            nop.sync_info = mybir.SyncInfo(
                                on_wait=extra[k : k + max_waits], on_update=[]
                            )
                            out.append(nop)
                        inst.sync_info = mybir.SyncInfo(
                            on_wait=waits[-max_waits:],
                            on_update=list(si.on_update or []),
                        )
                        changed = True
                out.append(inst)
            if changed:
                bb.instructions = out
    return nc


def _build_nc(reps=1, dedup=True):
    key = ("nc", reps, dedup, SKIP_DOTS, SKIP_SOFTMAX, CTX_REUSE)
    if key in _cache:
        return _cache[key]

    import concourse.bass as bass
    import concourse.tile as tile
    from concourse import mybir

    _patch_tile_framework()

    f32 = mybir.dt.float32
    bf = mybir.dt.bfloat16
    X = mybir.AxisListType.X
    XY = mybir.AxisListType.XY
    ADD = mybir.AluOpType.add
    MAX = mybir.AluOpType.max

    nc = bass.Bass(target_bir_lowering=False)

    hid_d = nc.dram_tensor("hidT", [128, KC, R], bf, kind="ExternalInput")
    ctx_d = nc.dram_tensor(
        "ctxT", [W - 1, NT, 128, KC, 128], bf, kind="ExternalInput"
    )
    wku_d = nc.dram_tensor(
        "wkuT", [PASSES, 128, KC, CW + 2 * UC], bf, kind="ExternalInput"
    )
    wq_d = nc.dram_tensor("wqT", [PASSES, KC, 128, CW], bf, kind="ExternalInput")
    out_d = nc.dram_tensor("qout", [R, A], f32, kind="ExternalOutput")

    qscale = 1.0 / math.sqrt(D)

    with tile.TileContext(nc) as tc:
        with tc.tile_pool(name="outer", bufs=1) as outer:
            hid_sb = outer.tile([128, KC, R], bf, tag="hidT")
            nc.sync.dma_start(out=hid_sb, in_=hid_d[:, :, :])
            out_sbs = [
                outer.tile([128, A], f32, tag=f"out{t}", name=f"out{t}")
                for t in range(NT)
            ]
            vp_sbs = [
                outer.tile([128, NTOK, 2 * UC], f32, tag=f"vp{t}", name=f"vp{t}")
                for t in range(NT)
            ]

            for _rep in range(reps):
              for pp in range(PASSES):
                with (
                    tc.tile_pool(name=f"res{pp}", bufs=1) as res,
                    tc.tile_pool(name=f"wqs{pp}", bufs=6) as wqs,
                    tc.tile_pool(name=f"ctx{pp}", bufs=6) as ctxp,
                    tc.tile_pool(name=f"prod{pp}", bufs=3) as prodp,
                ):
                    wku_sb = res.tile([128, KC, CW + 2 * UC], bf, tag="wku")
                    for c4 in range(4):
                        # ACT hwdge ring: don't queue this 4MB stream ahead of
                        # the latency-critical wq chunk DMAs on the SP ring
                        nc.scalar.dma_start(
                            out=wku_sb[:, 4 * c4 : 4 * c4 + 4, :],
                            in_=wku_d[pp, :, 4 * c4 : 4 * c4 + 4, :],
                        )
                    q_sbs = [
                        res.tile([128, CW], f32, tag=f"q{t}", name=f"q{t}")
                        for t in range(NT)
                    ]
                    sc_sbs = [
                        res.tile([128, HPP, NTOK], f32, tag=f"sc{t}", name=f"sc{t}")
                        for t in range(NT)
                    ]

                    # ---- Q phase: q = hidden @ Wq.T (this pass's head half)
                    qps_ctx = tc.tile_pool(name=f"qps{pp}", bufs=NT, space="PSUM")
                    qps = qps_ctx.__enter__()
                    q_ps = [
                        qps.tile([128, CW], f32, tag="qps", name=f"qps{t}")
                        for t in range(NT)
                    ]
                    for c in range(KC):
                        wq_sb = wqs.tile([128, CW], bf, tag="wq")
                        nc.sync.dma_start(out=wq_sb, in_=wq_d[pp, c, :, :])
                        for t in range(NT):
                            lhs = hid_sb[:, c, t * 128 : (t + 1) * 128]
                            for b2 in range(CW // 512):
                                nc.tensor.matmul(
                                    q_ps[t][:, b2 * 512 : (b2 + 1) * 512],
                                    lhs,
                                    wq_sb[:, b2 * 512 : (b2 + 1) * 512],
                                    start=(c == 0),
                                    stop=(c == KC - 1),
                                )
                    for t in range(NT):
                        # PSUM -> SBUF, folding in the 1/sqrt(D) score scale
                        nc.scalar.activation(
                            out=q_sbs[t],
                            in_=q_ps[t],
                            func=mybir.ActivationFunctionType.Copy,
                            scale=qscale,
                        )
                    qps_ctx.__exit__(None, None, None)

                    # ---- KV phase: per (token, tile): k-proj + u-proj + dots
                    if CTX_REUSE:
                        tok_shared = res.tile([128, KC, 128], bf, tag="tokshared")
                        nc.sync.dma_start(
                            out=tok_shared, in_=ctx_d[0, 0, :, :, :]
                        )
                    with (
                        tc.tile_pool(name=f"kvps{pp}", bufs=3, space="PSUM") as kvps,
                        tc.tile_pool(name=f"vpps{pp}", bufs=2, space="PSUM") as vpps,
                    ):
                        for t in range(NT):
                            for j in range(NTOK):
                                if j == 0:
                                    tok = None
                                elif CTX_REUSE:
                                    tok = tok_shared
                                else:
                                    tok = ctxp.tile([128, KC, 128], bf, tag="ctx")
                                    nc.sync.dma_start(
                                        out=tok, in_=ctx_d[j - 1, t, :, :, :]
                                    )
                                kb = kvps.tile([128, CW], f32, tag="kb")
                                if pp == 0:
                                    vpp = vpps.tile(
                                        [128, 2 * UC], f32, tag="vpp", name="vpp"
                                    )
                                else:
                                    vpp = None
                                for c in range(KC):
                                    st = c == 0
                                    sp = c == KC - 1
                                    lhs = (
                                        hid_sb[:, c, t * 128 : (t + 1) * 128]
                                        if j == 0
                                        else tok[:, c, :]
                                    )
                                    for b2 in range(CW // 512):
                                        nc.tensor.matmul(
                                            kb[:, b2 * 512 : (b2 + 1) * 512],
                                            lhs,
                                            wku_sb[:, c, b2 * 512 : (b2 + 1) * 512],
                                            start=st,
                                            stop=sp,
                                        )
                                    if pp == 0:
                                        nc.tensor.matmul(
                                            vpp,
                                            lhs,
                                            wku_sb[:, c, CW : CW + 2 * UC],
                                            start=st,
                                            stop=sp,
                                        )
                                if not SKIP_DOTS:
                                    pr = prodp.tile([128, CW], f32, tag="pr")
                                    nc.vector.tensor_mul(pr, kb, q_sbs[t])
                                    nc.vector.tensor_reduce(
                                        out=sc_sbs[t][:, :, j],
                                        in_=pr.rearrange("p (h d) -> p h d", d=D),
                                        axis=X,
                                        op=ADD,
                                    )
                                else:
                                    nc.vector.tensor_reduce(
                                        out=sc_sbs[t][:, :, j],
                                        in_=kb.rearrange("p (h d) -> p h d", d=D)[:, :, 0:1],
                                        axis=X,
                                        op=ADD,
                                    )
                                if pp == 0:
                                    nc.scalar.activation(
                                        out=vp_sbs[t][:, j, :],
                                        in_=vpp,
                                        func=mybir.ActivationFunctionType.Copy,
                                    )

                    # ---- softmax + combine per tile
                    with tc.tile_pool(name=f"sm{pp}", bufs=2) as smp:
                        for t in range(NT if not SKIP_SOFTMAX else 0):
                            mx = smp.tile([128, HPP], f32, tag=f"m{t}")
                            nc.vector.tensor_reduce(
                                out=mx, in_=sc_sbs[t], axis=X, op=MAX
                            )
                            et = smp.tile([128, HPP, NTOK], f32, tag=f"e{t}")
                            for j in range(NTOK):
                                nc.vector.tensor_sub(
                                    et[:, :, j], sc_sbs[t][:, :, j], mx
                                )
                            nc.scalar.activation(
                                out=et, in_=et, func=mybir.ActivationFunctionType.Exp
                            )
                            s8 = smp.tile([128, HPP], f32, tag=f"s8{t}")
                            nc.vector.tensor_reduce(out=s8, in_=et, axis=X, op=ADD)
                            # hidden token appears twice in the kv list
                            nc.vector.tensor_add(s8, s8, et[:, :, 0])
                            rcp = smp.tile([128, HPP], f32, tag=f"r{t}")
                            nc.vector.reciprocal(rcp, s8)
                            at = smp.tile([128, HPP, NTOK], f32, tag=f"a{t}")
                            for j in range(NTOK):
                                nc.vector.tensor_mul(at[:, :, j], et[:, :, j], rcp)
                            vv = vp_sbs[t].rearrange(
                                "p j (h a) -> p h j a", a=A
                            )[:, pp * HPP : (pp + 1) * HPP]
                            for a in range(A):
                                tmp = smp.tile([128, HPP, NTOK], f32, tag=f"tm{t}")
                                nc.vector.tensor_mul(tmp, at, vv[:, :, :, a])
                                r1 = smp.tile([128, 1], f32, tag=f"r1{t}")
                                r2 = smp.tile([128, 1], f32, tag=f"r2{t}")
                                nc.vector.tensor_reduce(
                                    out=r1, in_=tmp, axis=XY, op=ADD
                                )
                                nc.vector.tensor_reduce(
                                    out=r2, in_=tmp[:, :, 0], axis=X, op=ADD
                                )
                                nc.vector.tensor_add(r1, r1, r2)
                                if pp == 0:
                                    nc.vector.tensor_copy(
                                        out=out_sbs[t][:, a : a + 1], in_=r1
                                    )
                                else:
                                    nc.vector.tensor_add(
                                        out_sbs[t][:, a : a + 1],
                                        out_sbs[t][:, a : a + 1],
                                        r1,
                                    )

            for t in range(NT):
                nc.sync.dma_start(
                    out=out_d[t * 128 : (t + 1) * 128, :], in_=out_sbs[t]
                )

    if dedup:
        _dedup_ldweights(nc)
    _split_waits(nc)
    _cache[key] = nc
    return nc


def _prep_inputs(hidden_state, context_buffer, w_qkv, w_out, b_out, context_ptr):
    """Host-side sharding + layout: bf16 casts, partition-major transposes,
    w_out folded into Wv."""
    hidden_state = np.ascontiguousarray(hidden_state, dtype=np.float32)
    context_buffer = np.ascontiguousarray(context_buffer, dtype=np.float32)
    w_qkv = np.ascontiguousarray(w_qkv, dtype=np.float32)
    w_out = np.ascontiguousarray(w_out, dtype=np.float32)

    ptr = int(context_ptr) % W
    kept = [w for w in range(W) if w != ptr]

    wq = w_qkv[0:H]
    wk = w_qkv[H : 2 * H]
    wv = w_qkv[2 * H : 3 * H]
    # U[(h*A+a), ci] = sum_d w_out[a, h*D+d] * Wv[h*D+d, ci]
    U = (
        np.einsum(
            "ahd,hdc->hac",
            w_out.reshape(A, NH, D).astype(np.float64),
            wv.reshape(NH, D, H).astype(np.float64),
        )
        .reshape(NH * A, H)
        .astype(np.float32)
    )

    # weights, partition-major per pass
    wkuT = np.zeros((PASSES, 128, KC, CW + 2 * UC), dtype=BF16)
    wqT = np.empty((PASSES, KC, 128, CW), dtype=BF16)
    wkT = wk.T.reshape(KC, 128, H)  # [c, p, n]
    uT = U.T.reshape(KC, 128, NH * A)
    wqTf = wq.T.reshape(KC, 128, H)
    for p_ in range(PASSES):
        wkuT[p_, :, :, 0:CW] = (
            wkT[:, :, p_ * CW : (p_ + 1) * CW].transpose(1, 0, 2).astype(BF16)
        )
        wqT[p_] = wqTf[:, :, p_ * CW : (p_ + 1) * CW].astype(BF16)
    wkuT[0, :, :, CW : CW + 2 * UC] = uT.transpose(1, 0, 2).astype(BF16)
    wkuT = np.ascontiguousarray(wkuT)
    wqT = np.ascontiguousarray(wqT)

    in_maps = []
    for core in range(NCORES):
        rows = slice(core * R, (core + 1) * R)
        # hidT [p, c, r]
        hidT = np.ascontiguousarray(
            hidden_state[rows].T.reshape(KC, 128, R).transpose(1, 0, 2)
        ).astype(BF16)
        # ctxT [j, t, p, c, r128]
        ctx = context_buffer[rows][:, kept, :]  # [R, 7, H]
        ctxT = np.ascontiguousarray(
            ctx.transpose(1, 2, 0)  # [7, H, R]
            .reshape(W - 1, KC, 128, NT, 128)
            .transpose(0, 3, 2, 1, 4)  # [j, t, p, c, r]
        ).astype(BF16)
        in_maps.append(dict(hidT=hidT, ctxT=ctxT, wkuT=wkuT, wqT=wqT))
    return in_maps


def kernel(hidden_state, context_buffer, w_qkv, w_out, b_out, context_ptr):
    from concourse.bass_utils import run_bass_kernel_spmd

    nc = _build_nc()
    in_maps = _prep_inputs(
        hidden_state, context_buffer, w_qkv, w_out, b_out, context_ptr
    )
    res = run_bass_kernel_spmd(nc, in_maps, core_ids=list(range(NCORES)))
    out = np.concatenate([r["qout"] for r in res.results], axis=0)
    return (out + np.asarray(b_out, dtype=np.float32)[None, :]).astype(np.float32)
